# revision 29
# baseline (speedup 1.0000x reference)
"""EnhancedGAT Trainium2 kernel: 8-core SPMD, node-sharded edge phase.

Design:
- Nodes padded to NPAD (multiple of 8*128); core c owns dst nodes
  [c*NC_NODES, (c+1)*NC_NODES), processed as TILES tiles of 128 dst nodes.
- Edges (+self loops) sorted by dst, assigned to the owning core/tile,
  split per tile into lo/hi halves by src (< NSPLIT, chosen so the lo
  half fits one <=1024-index dma_gather) so gather indices fit int16,
  padded to uniform chunk counts CH_LO/CH_HI across all cores/tiles.
- Layer-0 node table (dense phase of layer 0) is computed on the host
  and shipped as an input; layers 1/2 compute own table rows fused into
  the previous edge phase's tile loop, then AllGather.
- Feature columns are kept in f-major (f*H+h) order end-to-end so the
  per-edge softmax-weight scaling is a packed-last-dim DVE op (2x mode).
- Edge phase per tile: dma_gather of src rows; one-hot B[e,(d,c)] built
  on DVE in c-last layout (2x mode); A[d,(c,e)] = per-chunk PE
  transposes of B evacuated by the Activation engine; per-chunk one-hot
  matmuls scatter w*h and w into PSUM; e_dst per edge via small PE
  matmuls with A; leaky-relu/exp on the Activation engine.
"""

import sys

sys.path.insert(0, "/opt/trn_rl_repo")

import numpy as np
import ml_dtypes

BF16 = ml_dtypes.bfloat16
NEG_SLOPE = 0.2
BN_EPS = 1e-5
P = 128
ROW = 256  # bf16 cols per table row (512B)
GB = 24  # chunks per u-add group
GMAX = 8  # max chunks per dma_gather call (1024-descriptor hw limit)


def full_cfg():
    return dict(
        n_cores=8, n=50000, e=800000, fin=128, h=4, fh=32, hf=128, mlp=64
    )


def derive_cfg(cfg):
    c = dict(cfg)
    n_cores = c["n_cores"]
    npad = ((c["n"] + n_cores * P - 1) // (n_cores * P)) * (n_cores * P)
    c["npad"] = npad
    c["nc_nodes"] = npad // n_cores
    c["tiles"] = c["nc_nodes"] // P
    return c


# ---------------------------------------------------------------- host side


def _wrap_idx(flat):
    """int16 gather-index layout: flat[i] lives at wrapped[i%16, i//16],
    replicated to 128 partitions."""
    s = len(flat) // 16
    w = flat.reshape(s, 16).T.astype(np.int16)  # [16, s]
    return np.tile(w, (P // 16, 1))  # [128, s]


def preprocess_edges(edge_index, cfg):
    """Returns per-core index arrays + updates cfg with CH_LO/CH_HI."""
    n, npad, n_cores = cfg["n"], cfg["npad"], cfg["n_cores"]
    nc_nodes, tiles = cfg["nc_nodes"], cfg["tiles"]
    src = np.concatenate([edge_index[0], np.arange(n, dtype=np.int64)])
    dst = np.concatenate([edge_index[1], np.arange(n, dtype=np.int64)])
    order = np.argsort(dst, kind="stable")
    src_s = src[order].astype(np.int32)
    dst_s = dst[order].astype(np.int32)

    # pick the lo/hi split (both halves must index with int16, i.e. be
    # <= 32768 rows) minimizing dma_gather calls then total chunks
    bounds = np.searchsorted(dst_s, np.arange(0, npad + 1, P))
    nblk = npad // P
    ntile = n_cores * tiles
    tile_of_edge = np.searchsorted(bounds, np.arange(len(dst_s)), "right") - 1
    srcblk = src_s // P
    hist = np.zeros((ntile, nblk + 1), np.int64)
    np.add.at(hist, (tile_of_edge, srcblk), 1)
    pref = np.cumsum(hist, axis=1)  # pref[ti, k] = edges with src < (k+1)*P
    tot = pref[:, -1]
    lo_blk = max(1, (npad - 32768) // P)
    hi_blk = min(32768 // P, nblk)
    best = None
    for k in range(lo_blk, hi_blk + 1):
        max_lo = int(pref[:, k - 1].max())
        max_hi = int((tot - pref[:, k - 1]).max())
        cl = max(1, -(-max_lo // P))
        ch = max(1, -(-max_hi // P))
        key = (-(-cl // GMAX) + -(-ch // GMAX), cl + ch)
        if best is None or key < best[0]:
            best = (key, k * P, cl, ch)
    _, nsplit, ch_lo, ch_hi = best
    cfg["nsplit"] = nsplit

    per_ct = []  # (src_lo, dloc_lo, src_hi, dloc_hi) per (core,tile)
    for c in range(n_cores):
        for t in range(tiles):
            ti = c * tiles + t
            a, b = bounds[ti], bounds[ti + 1]
            srcs = src_s[a:b]
            dloc = (dst_s[a:b] - ti * P).astype(np.int32)
            m = srcs < nsplit
            sl, dl = srcs[m], dloc[m]
            sh, dh = srcs[~m] - nsplit, dloc[~m]
            per_ct.append((sl, dl, sh, dh))
    cht = ch_lo + ch_hi
    cfg["ch_lo"], cfg["ch_hi"], cfg["cht"] = ch_lo, ch_hi, cht

    w_lo, w_hi = ch_lo * 8, ch_hi * 8
    wt = w_lo + w_hi
    ix_all = np.zeros((n_cores, P, tiles * wt), np.int16)
    dcol_all = np.full((n_cores, P, tiles * cht), -1.0, np.float32)
    for c in range(n_cores):
        for t in range(tiles):
            sl, dl, sh, dh = per_ct[c * tiles + t]
            fl = np.zeros(ch_lo * P, np.int32)
            fl[: len(sl)] = sl
            fhh = np.zeros(ch_hi * P, np.int32)
            fhh[: len(sh)] = sh
            ix_all[c, :, t * wt : t * wt + w_lo] = _wrap_idx(fl)
            ix_all[c, :, t * wt + w_lo : (t + 1) * wt] = _wrap_idx(fhh)
            # dcol[p, c] = dst-local index of edge slot (chunk c, lane p)
            dloc_flat = np.full(cht * P, -1.0, np.float32)
            dloc_flat[: len(dl)] = dl
            dloc_flat[ch_lo * P : ch_lo * P + len(dh)] = dh
            dcol_all[c, :, t * cht : (t + 1) * cht] = dloc_flat.reshape(
                cht, P
            ).T
    return ix_all, dcol_all.astype(BF16)


def feat_perm(cfg):
    """f-major feature order: new col f*h_+h holds old col h*fh+f."""
    h_, fh = cfg["h"], cfg["fh"]
    j = np.arange(h_ * fh)
    return (j % h_) * fh + j // h_


def fold_weights(inp, cfg):
    """Host-folded constant tensors (shared across cores)."""
    h_, fh, hf, mlp = cfg["h"], cfg["fh"], cfg["hf"], cfg["mlp"]
    cht = cfg["cht"]
    perm = feat_perm(cfg)

    def wa(W, a):
        # Wa[k, head] = sum_f W[k, head*fh+f] * a[head, f]
        return np.einsum("khf,hf->kh", W.reshape(-1, h_, fh), a)

    def colp(Wa):
        # permute the feature block of the 136 columns to f-major
        return np.concatenate([Wa[:, :hf][:, perm], Wa[:, hf:]], 1)

    W1, W2, W3 = inp["W1"], inp["W2"], inp["W3"]
    s1 = inp["g1"] / np.sqrt(1.0 + BN_EPS)
    t1 = inp["be1"]
    s2 = inp["g2"] / np.sqrt(1.0 + BN_EPS)
    t2 = inp["be2"]
    W2a_raw = np.concatenate([W2, wa(W2, inp["a2s"]), wa(W2, inp["a2d"])], 1)
    W2a = colp(W2a_raw * s2[:, None])
    cshift1 = colp((t2 @ W2a_raw)[None, :])[0]  # [136]
    W3a = colp(
        np.concatenate([W3, wa(W3, inp["a3s"]), wa(W3, inp["a3d"])], 1)
    )[perm, :]  # rows: layer-1 activations are f-major
    M2wp = inp["M2w"] * s1[:, None]
    mlpb2 = inp["M2b"] + t1 @ inp["M2w"]

    def bc(v, rows=P):
        return np.tile(np.asarray(v, np.float32)[None, :], (rows, 1))

    consts = dict(
        W2a=W2a.astype(BF16),
        W3a=W3a.astype(BF16),
        M1w=inp["M1w"][perm, :].astype(BF16),
        M2wp=M2wp.astype(BF16),
        b1_bc=bc(inp["b1"][perm]).astype(BF16),
        b2_bc=bc(inp["b2"][perm]).astype(BF16),
        b3_bc=bc(inp["b3"]),
        m1b_r=inp["M1b"][None, :].astype(BF16),
        m2b_r=mlpb2[None, :].astype(BF16),
        csh1_r=cshift1[None, :].astype(BF16),
        ones_r=np.ones((1, P), np.float32).astype(BF16),
        identf=np.eye(P, dtype=np.float32),
        dmaj=np.repeat(np.arange(P, dtype=np.float32), cht)[None, :]
        .repeat(P, 0)
        .astype(BF16),
        ident=np.eye(P, dtype=np.float32).astype(BF16),
    )
    return consts


# ---------------------------------------------------------------- program


def build_program(cfg):
    import concourse.bacc as bacc
    import concourse.mybir as mybir
    import concourse.tile as tile

    fp32 = mybir.dt.float32
    bf16 = mybir.dt.bfloat16
    i16 = mybir.dt.int16
    AF = mybir.ActivationFunctionType
    OP = mybir.AluOpType

    n_cores = cfg["n_cores"]
    npad, nc_nodes, tiles = cfg["npad"], cfg["nc_nodes"], cfg["tiles"]
    nsplit = cfg["nsplit"]
    ch_lo, ch_hi, cht = cfg["ch_lo"], cfg["ch_hi"], cfg["cht"]
    hf, mlp, h_, fh = cfg["hf"], cfg["mlp"], cfg["h"], cfg["fh"]
    ecols = hf + 2 * h_  # 136
    w_lo, w_hi = ch_lo * 8, ch_hi * 8
    wt = w_lo + w_hi
    ah = -(-cht // 2)  # chunks per A-transpose half

    nc = bacc.Bacc("TRN2", target_bir_lowering=False, debug=False)

    # ---- I/O
    table0_d = nc.dram_tensor("table0", [npad, ROW], bf16, kind="ExternalInput")
    ix_d = nc.dram_tensor("ix_all", [P, tiles * wt], i16, kind="ExternalInput")
    dcol_d = nc.dram_tensor(
        "dcol_all", [P, tiles * cht], bf16, kind="ExternalInput"
    )
    edst0_d = nc.dram_tensor(
        "edst0", [P, tiles * h_], bf16, kind="ExternalInput"
    )
    cn = {}
    cshapes = dict(
        W2a=[P, ecols], W3a=[P, ecols], M1w=[P, mlp], M2wp=[mlp, hf],
        b1_bc=[P, hf], b2_bc=[P, hf], b3_bc=[P, fh],
        m1b_r=[1, mlp], m2b_r=[1, hf], csh1_r=[1, ecols], ones_r=[1, P],
        dmaj=[P, P * cht], ident=[P, P], identf=[P, P],
    )
    cdt = dict(b3_bc=fp32, identf=fp32)
    for k, shp in cshapes.items():
        cn[k] = nc.dram_tensor(k, shp, cdt.get(k, bf16), kind="ExternalInput")
    out_d = nc.dram_tensor("out", [nc_nodes, fh], fp32, kind="ExternalOutput")

    # ---- internal DRAM
    aspace = "Shared" if n_cores > 4 else "Local"
    table = [
        table0_d,
        nc.dram_tensor("table1", [npad, ROW], bf16, addr_space=aspace),
        nc.dram_tensor("table2", [npad, ROW], bf16, addr_space=aspace),
    ]
    ag_in = [
        None,
        nc.dram_tensor("ag_in1", [nc_nodes, ROW], bf16),
        nc.dram_tensor("ag_in2", [nc_nodes, ROW], bf16),
    ]

    with tile.TileContext(nc) as tc:
        with (
            tc.tile_pool(name="const", bufs=1) as cpool,
            tc.tile_pool(name="work", bufs=4) as wpool,
            tc.tile_pool(name="gath", bufs=3) as gpool,
            tc.tile_pool(name="onehot", bufs=3) as opool,
            tc.tile_pool(name="psum", bufs=2, space="PSUM") as ppool,
            tc.tile_pool(name="psumd", bufs=2, space="PSUM") as pdpool,
            tc.tile_pool(name="psumA", bufs=1, space="PSUM") as papool,
        ):
            C = {}
            for k, shp in cshapes.items():
                tl = cpool.tile(shp, cdt.get(k, bf16), tag=f"c_{k}")
                nc.sync.dma_start(out=tl[:], in_=cn[k][:, :])
                C[k] = tl

            # persistent per-layer-invariant buffers (loaded once)
            ix_sb = cpool.tile([P, tiles * wt], i16, tag="ix_sb")
            nc.sync.dma_start(out=ix_sb[:], in_=ix_d[:, :])
            dcol_sb = cpool.tile([P, tiles * cht], bf16, tag="dcol_sb")
            nc.sync.dma_start(out=dcol_sb[:], in_=dcol_d[:, :])
            edst_sb0 = cpool.tile([P, tiles * h_], bf16, tag="edst0")
            edst_sb1 = cpool.tile([P, tiles * h_], bf16, tag="edst1")
            edst_sb2 = cpool.tile([P, tiles * h_], bf16, tag="edst2")
            edst_sb = [edst_sb0, edst_sb1, edst_sb2]
            nc.sync.dma_start(out=edst_sb[0][:], in_=edst0_d[:, :])

            dmaj3 = C["dmaj"][:].rearrange("p (d c) -> p d c", c=cht)

            def fused_layer(layer):
                """Edge phase of `layer` + fused dense phase of layer+1."""
                tbl = table[layer]
                write_out = layer == 2
                for t in range(tiles):
                    gat = gpool.tile([P, cht * ROW], bf16, tag="gat")
                    g3 = gat[:].rearrange("p (c r) -> p c r", r=ROW)

                    def emit_gathers(base_c, nch_half, tbl_ap, ixoff):
                        done = 0
                        while done < nch_half:
                            gsz = min(GMAX, nch_half - done)
                            nc.gpsimd.dma_gather(
                                out_ap=g3[
                                    :, base_c + done : base_c + done + gsz, :
                                ],
                                in_ap=tbl_ap,
                                idxs_ap=ix_sb[
                                    :,
                                    ixoff + done * 8 : ixoff + (done + gsz) * 8,
                                ],
                                num_idxs=gsz * P,
                                num_idxs_reg=gsz * P,
                                elem_size=ROW,
                            )
                            done += gsz

                    emit_gathers(0, ch_lo, tbl[0:nsplit, :], t * wt)
                    emit_gathers(
                        ch_lo, ch_hi, tbl[nsplit:npad, :], t * wt + w_lo
                    )
                    # one-hot B[e, (d,c)]: c-last layout keeps every AP's
                    # last dim packed -> DVE 2x mode
                    B = opool.tile([P, P * cht], bf16, tag="B")
                    B3 = B[:].rearrange("p (d c) -> p d c", c=cht)
                    nc.vector.tensor_tensor(
                        out=B3,
                        in0=dcol_sb[:, t * cht : (t + 1) * cht][
                            :, None, :
                        ].to_broadcast([P, P, cht]),
                        in1=dmaj3,
                        op=OP.is_equal,
                    )
                    # A[d, (c,e)] = B^T per chunk, via PE transposes
                    # evacuated from PSUM by the Activation engine
                    A = opool.tile([P, cht * P], bf16, tag="A")
                    for half in range(2):
                        c0, c1 = half * ah, min((half + 1) * ah, cht)
                        pT = papool.tile([P, ah * P], bf16, tag="pT")
                        for c in range(c0, c1):
                            nc.tensor.transpose(
                                out=pT[:, (c - c0) * P : (c - c0 + 1) * P],
                                in_=B3[:, :, c],
                                identity=C["ident"][:],
                            )
                        nc.scalar.activation(
                            out=A[:, c0 * P : c1 * P],
                            in_=pT[:, : (c1 - c0) * P],
                            func=AF.Copy,
                        )
                    # e_dst expansion + leaky_relu + exp -> w in esrc slot
                    edt = edst_sb[layer][:, t * h_ : (t + 1) * h_]
                    ngrp = -(-cht // GB)
                    for g in range(ngrp):
                        c0, c1 = g * GB, min((g + 1) * GB, cht)
                        nch = c1 - c0
                        pex = ppool.tile([P, GB * h_], fp32, tag="pexp")
                        for c in range(c0, c1):
                            nc.tensor.matmul(
                                pex[:, (c - c0) * h_ : (c - c0 + 1) * h_],
                                lhsT=A[:, c * P : (c + 1) * P],
                                rhs=edt,
                                start=True,
                                stop=True,
                            )
                        u = wpool.tile([P, GB * h_], fp32, tag="u")
                        nc.vector.tensor_tensor(
                            out=u[:, : nch * h_],
                            in0=pex[:, : nch * h_],
                            in1=g3[:, c0:c1, hf : hf + h_],
                            op=OP.add,
                        )
                        ul = wpool.tile([P, GB * h_], fp32, tag="ul")
                        nc.scalar.activation(
                            out=ul[:, : nch * h_],
                            in_=u[:, : nch * h_],
                            func=AF.Lrelu,
                            alpha=NEG_SLOPE,
                        )
                        nc.scalar.activation(
                            out=g3[:, c0:c1, hf : hf + h_],
                            in_=ul[:, : nch * h_],
                            func=AF.Exp,
                        )
                    # w-scale features in place (f-major rows: h is the
                    # packed last dim so the broadcast stays in 2x mode)
                    nc.vector.tensor_tensor(
                        out=g3[:, :, 0:hf].rearrange(
                            "p c (f h) -> p c f h", h=h_
                        ),
                        in0=g3[:, :, 0:hf].rearrange(
                            "p c (f h) -> p c f h", h=h_
                        ),
                        in1=g3[:, :, hf : hf + h_][:, :, None, :].to_broadcast(
                            [P, cht, fh, h_]
                        ),
                        op=OP.mult,
                    )
                    # scatter w*h (and w, for the softmax denominator)
                    pm = ppool.tile([P, hf + h_], fp32, tag="pmain")
                    for c in range(cht):
                        nc.tensor.matmul(
                            pm[:],
                            lhsT=B3[:, :, c],
                            rhs=g3[:, c, 0 : hf + h_],
                            start=(c == 0),
                            stop=(c == cht - 1),
                        )
                    # epilogue: normalize
                    zr = wpool.tile([P, h_], fp32, tag="zr")
                    nc.vector.tensor_scalar(
                        out=zr[:], in0=pm[:, hf : hf + h_], scalar1=1e-16,
                        scalar2=None, op0=OP.add,
                    )
                    rec = wpool.tile([P, h_], fp32, tag="rec")
                    nc.vector.reciprocal(out=rec[:], in_=zr[:])
                    if write_out:
                        nc.vector.tensor_scalar(
                            out=rec[:], in0=rec[:], scalar1=1.0 / h_,
                            scalar2=None, op0=OP.mult,
                        )
                        fn = wpool.tile([P, hf], fp32, tag="fnw")
                        nc.vector.tensor_tensor(
                            out=fn[:].rearrange("p (f h) -> p f h", h=h_),
                            in0=pm[:, 0:hf].rearrange("p (f h) -> p f h", h=h_),
                            in1=rec[:, None, :].to_broadcast([P, fh, h_]),
                            op=OP.mult,
                        )
                        hm = wpool.tile([P, fh], fp32, tag="hm")
                        nc.vector.tensor_reduce(
                            out=hm[:],
                            in_=fn[:].rearrange("p (f h) -> p f h", h=h_),
                            axis=mybir.AxisListType.X,
                            op=OP.add,
                        )
                        ob = wpool.tile([P, fh], fp32, tag="ob")
                        nc.vector.tensor_tensor(
                            out=ob[:], in0=hm[:], in1=C["b3_bc"][:], op=OP.add
                        )
                        nc.sync.dma_start(
                            out=out_d[t * P : (t + 1) * P, :], in_=ob[:]
                        )
                        continue
                    fn = wpool.tile([P, hf], bf16, tag="fn")
                    nc.vector.tensor_tensor(
                        out=fn[:].rearrange("p (f h) -> p f h", h=h_),
                        in0=pm[:, 0:hf].rearrange("p (f h) -> p f h", h=h_),
                        in1=rec[:, None, :].to_broadcast([P, fh, h_]),
                        op=OP.mult,
                    )
                    bb = C["b1_bc"] if layer == 0 else C["b2_bc"]
                    nc.vector.tensor_tensor(
                        out=fn[:], in0=fn[:], in1=bb[:], op=OP.add
                    )
                    xo = wpool.tile([P, hf], fp32, tag="xo")
                    nc.scalar.activation(out=xo[:], in_=fn[:], func=AF.Relu)

                    # ---- fused dense phase for layer+1 on this tile
                    # (dense PSUM stages rotate through one fp32 tag)
                    pxt = pdpool.tile([P, ecols], fp32, tag="pd")
                    nc.tensor.transpose(
                        out=pxt[:, 0:P], in_=xo[:], identity=C["identf"][:]
                    )
                    xoT = wpool.tile([P, P], bf16, tag="xoT")
                    nc.vector.tensor_copy(out=xoT[:], in_=pxt[:, 0:P])
                    if layer == 0:
                        p1 = pdpool.tile([P, ecols], fp32, tag="pd")
                        nc.tensor.matmul(
                            p1[:, 0:mlp], lhsT=C["ones_r"][:],
                            rhs=C["m1b_r"][:], start=True, stop=False,
                        )
                        nc.tensor.matmul(
                            p1[:, 0:mlp], lhsT=xoT[:], rhs=C["M1w"][:],
                            start=False, stop=True,
                        )
                        r1 = wpool.tile([P, mlp], fp32, tag="r1")
                        nc.scalar.activation(
                            out=r1[:], in_=p1[:, 0:mlp], func=AF.Relu
                        )
                        pr = pdpool.tile([P, ecols], fp32, tag="pd")
                        nc.tensor.transpose(
                            out=pr[0:mlp, 0:P], in_=r1[:],
                            identity=C["identf"][:],
                        )
                        r1T = wpool.tile([mlp, P], bf16, tag="r1T")
                        nc.scalar.activation(
                            out=r1T[:], in_=pr[0:mlp, 0:P], func=AF.Copy
                        )
                        p2 = pdpool.tile([P, ecols], fp32, tag="pd")
                        nc.tensor.matmul(
                            p2[:, 0:hf], lhsT=C["ones_r"][:],
                            rhs=C["m2b_r"][:], start=True, stop=False,
                        )
                        nc.tensor.matmul(
                            p2[:, 0:hf], lhsT=r1T[:], rhs=C["M2wp"][:],
                            start=False, stop=True,
                        )
                        y2 = wpool.tile([P, hf], fp32, tag="y2")
                        nc.scalar.activation(
                            out=y2[:], in_=p2[:, 0:hf], func=AF.Relu
                        )
                        py = pdpool.tile([P, ecols], fp32, tag="pd")
                        nc.tensor.transpose(
                            out=py[:, 0:P], in_=y2[:], identity=C["identf"][:]
                        )
                        y2T = wpool.tile([P, P], bf16, tag="y2T")
                        nc.scalar.activation(
                            out=y2T[:], in_=py[:, 0:P], func=AF.Copy
                        )
                        pd = pdpool.tile([P, ecols], fp32, tag="pd")
                        nc.tensor.matmul(
                            pd[:], lhsT=C["ones_r"][:], rhs=C["csh1_r"][:],
                            start=True, stop=False,
                        )
                        nc.tensor.matmul(
                            pd[:], lhsT=y2T[:], rhs=C["W2a"][:],
                            start=False, stop=True,
                        )
                    else:
                        pd = pdpool.tile([P, ecols], fp32, tag="pd")
                        nc.tensor.matmul(
                            pd[:], lhsT=xoT[:], rhs=C["W3a"][:],
                            start=True, stop=True,
                        )
                    stg = wpool.tile([P, ecols], bf16, tag="stg")
                    nc.vector.tensor_copy(out=stg[:], in_=pd[:])
                    nc.vector.tensor_copy(
                        out=edst_sb[layer + 1][:, t * h_ : (t + 1) * h_],
                        in_=stg[:, hf + h_ : hf + 2 * h_],
                    )
                    nc.sync.dma_start(
                        out=ag_in[layer + 1][t * P : (t + 1) * P, 0:ecols],
                        in_=stg[:],
                    )
                if not write_out:
                    nc.gpsimd.collective_compute(
                        "AllGather",
                        mybir.AluOpType.bypass,
                        ins=[ag_in[layer + 1].ap().opt()],
                        outs=[table[layer + 1].ap().opt()],
                        replica_groups=[list(range(n_cores))],
                    )

            fused_layer(0)
            fused_layer(1)
            fused_layer(2)

    nc.compile()
    return nc


# ---------------------------------------------------------------- kernel()

_CACHE = {}


def make_in_maps(inputs, cfg):
    n, npad, n_cores = cfg["n"], cfg["npad"], cfg["n_cores"]
    nc_nodes, tiles = cfg["nc_nodes"], cfg["tiles"]
    h_ = cfg["h"]
    inp = {k: np.asarray(v) for k, v in inputs.items()}
    ix_all, dcol_all = preprocess_edges(inp["edge_index"], cfg)
    consts = fold_weights(inp, cfg)
    xpad = np.zeros((npad, cfg["fin"]), np.float32)
    xpad[:n] = inp["x"]

    def wa(W, a):
        return np.einsum(
            "khf,hf->kh", W.reshape(-1, cfg["h"], cfg["fh"]), a
        )

    # host-computed layer-0 node table: [h (f-major) | e_src | e_dst | 0-pad]
    hf = cfg["hf"]
    table0 = np.zeros((npad, ROW), np.float32)
    table0[:, :hf] = (xpad @ inp["W1"])[:, feat_perm(cfg)]
    table0[:, hf : hf + h_] = xpad @ wa(inp["W1"], inp["a1s"])
    table0[:, hf + h_ : hf + 2 * h_] = xpad @ wa(inp["W1"], inp["a1d"])
    table0 = table0.astype(BF16)
    in_maps = []
    for c in range(n_cores):
        # edst0 in [P, tiles*h] partition-major layout for own nodes
        ed = table0[c * nc_nodes : (c + 1) * nc_nodes, hf + h_ : hf + 2 * h_]
        ed_pm = np.ascontiguousarray(
            ed.reshape(tiles, P, h_).transpose(1, 0, 2).reshape(P, tiles * h_)
        )
        m = dict(
            table0=table0,
            ix_all=ix_all[c],
            dcol_all=dcol_all[c],
            edst0=ed_pm,
        )
        for k, v in consts.items():
            m[k] = v
        in_maps.append(m)
    return in_maps


def kernel(**inputs):
    from concourse import bass_utils

    cfg = derive_cfg(full_cfg())
    in_maps = make_in_maps(inputs, cfg)
    key = ("prog", cfg["ch_lo"], cfg["ch_hi"], cfg["nsplit"])
    if key not in _CACHE:
        _CACHE[key] = build_program(cfg)
    nc = _CACHE[key]
    res = bass_utils.run_bass_kernel_spmd(
        nc, in_maps, core_ids=list(range(cfg["n_cores"]))
    )
    outs = [res.results[c]["out"] for c in range(cfg["n_cores"])]
    full = np.concatenate(outs, axis=0)[: cfg["n"]]
    return full.astype(np.float32)


# revision 33
# speedup vs baseline: 1.2100x; 1.2100x over previous
"""EnhancedGAT Trainium2 kernel: 8-core SPMD, node-sharded edge phase.

Design:
- Nodes padded to NPAD (multiple of 8*128); core c owns dst nodes
  [c*NC_NODES, (c+1)*NC_NODES), processed as TILES tiles of 128 dst nodes.
- Edges (+self loops) sorted by dst, assigned to the owning core/tile,
  split per tile into lo/hi halves by src (< NSPLIT, chosen so the lo
  half fits one <=1024-index dma_gather) so gather indices fit int16,
  padded to uniform chunk counts CH_LO/CH_HI across all cores/tiles.
- Layer-0 node table (dense phase of layer 0) is computed on the host
  and shipped as an input; layers 1/2 compute own table rows fused into
  the previous edge phase's tile loop, then AllGather.
- Feature columns are kept in f-major (f*H+h) order end-to-end so the
  per-edge softmax-weight scaling is a packed-last-dim DVE op (2x mode).
- Edge phase per tile: dma_gather of src rows; one-hot B[e,(d,c)] built
  on DVE in c-last layout (2x mode); A[d,(c,e)] = per-chunk PE
  transposes of B evacuated by the Activation engine; per-chunk one-hot
  matmuls scatter w*h and w into PSUM; e_dst per edge via small PE
  matmuls with A; leaky-relu/exp on the Activation engine.
"""

import sys

sys.path.insert(0, "/opt/trn_rl_repo")

import numpy as np
import ml_dtypes

BF16 = ml_dtypes.bfloat16
NEG_SLOPE = 0.2
BN_EPS = 1e-5
P = 128
ROW = 256  # bf16 cols per table row (512B)
GB = 24  # chunks per u-add group
GMAX = 8  # max chunks per dma_gather call (1024-descriptor hw limit)


def full_cfg():
    return dict(
        n_cores=8, n=50000, e=800000, fin=128, h=4, fh=32, hf=128, mlp=64
    )


def derive_cfg(cfg):
    c = dict(cfg)
    n_cores = c["n_cores"]
    npad = ((c["n"] + n_cores * P - 1) // (n_cores * P)) * (n_cores * P)
    c["npad"] = npad
    c["nc_nodes"] = npad // n_cores
    c["tiles"] = c["nc_nodes"] // P
    return c


# ---------------------------------------------------------------- host side


def _wrap_idx(flat):
    """int16 gather-index layout: flat[i] lives at wrapped[i%16, i//16],
    replicated to 128 partitions."""
    s = len(flat) // 16
    w = flat.reshape(s, 16).T.astype(np.int16)  # [16, s]
    return np.tile(w, (P // 16, 1))  # [128, s]


def preprocess_edges(edge_index, cfg):
    """Returns per-core index arrays + updates cfg with CH_LO/CH_HI."""
    n, npad, n_cores = cfg["n"], cfg["npad"], cfg["n_cores"]
    nc_nodes, tiles = cfg["nc_nodes"], cfg["tiles"]
    src = np.concatenate([edge_index[0], np.arange(n, dtype=np.int64)])
    dst = np.concatenate([edge_index[1], np.arange(n, dtype=np.int64)])
    order = np.argsort(dst, kind="stable")
    src_s = src[order].astype(np.int32)
    dst_s = dst[order].astype(np.int32)

    # pick the lo/hi split (both halves must index with int16, i.e. be
    # <= 32768 rows) minimizing dma_gather calls then total chunks
    bounds = np.searchsorted(dst_s, np.arange(0, npad + 1, P))
    nblk = npad // P
    ntile = n_cores * tiles
    tile_of_edge = np.searchsorted(bounds, np.arange(len(dst_s)), "right") - 1
    srcblk = src_s // P
    hist = np.zeros((ntile, nblk + 1), np.int64)
    np.add.at(hist, (tile_of_edge, srcblk), 1)
    pref = np.cumsum(hist, axis=1)  # pref[ti, k] = edges with src < (k+1)*P
    tot = pref[:, -1]
    lo_blk = max(1, (npad - 32768) // P)
    hi_blk = min(32768 // P, nblk)
    best = None
    for k in range(lo_blk, hi_blk + 1):
        max_lo = int(pref[:, k - 1].max())
        max_hi = int((tot - pref[:, k - 1]).max())
        cl = max(1, -(-max_lo // P))
        ch = max(1, -(-max_hi // P))
        key = (-(-cl // GMAX) + -(-ch // GMAX), cl + ch)
        if best is None or key < best[0]:
            best = (key, k * P, cl, ch)
    _, nsplit, ch_lo, ch_hi = best
    cfg["nsplit"] = nsplit

    per_ct = []  # (src_lo, dloc_lo, src_hi, dloc_hi) per (core,tile)
    for c in range(n_cores):
        for t in range(tiles):
            ti = c * tiles + t
            a, b = bounds[ti], bounds[ti + 1]
            srcs = src_s[a:b]
            dloc = (dst_s[a:b] - ti * P).astype(np.int32)
            m = srcs < nsplit
            sl, dl = srcs[m], dloc[m]
            sh, dh = srcs[~m] - nsplit, dloc[~m]
            per_ct.append((sl, dl, sh, dh))
    cht = ch_lo + ch_hi
    cfg["ch_lo"], cfg["ch_hi"], cfg["cht"] = ch_lo, ch_hi, cht

    w_lo, w_hi = ch_lo * 8, ch_hi * 8
    wt = w_lo + w_hi
    ix_all = np.zeros((n_cores, P, tiles * wt), np.int16)
    dcol_all = np.full((n_cores, P, tiles * cht), -1.0, np.float32)
    for c in range(n_cores):
        for t in range(tiles):
            sl, dl, sh, dh = per_ct[c * tiles + t]
            fl = np.zeros(ch_lo * P, np.int32)
            fl[: len(sl)] = sl
            fhh = np.zeros(ch_hi * P, np.int32)
            fhh[: len(sh)] = sh
            ix_all[c, :, t * wt : t * wt + w_lo] = _wrap_idx(fl)
            ix_all[c, :, t * wt + w_lo : (t + 1) * wt] = _wrap_idx(fhh)
            # dcol[p, c] = dst-local index of edge slot (chunk c, lane p)
            dloc_flat = np.full(cht * P, -1.0, np.float32)
            dloc_flat[: len(dl)] = dl
            dloc_flat[ch_lo * P : ch_lo * P + len(dh)] = dh
            dcol_all[c, :, t * cht : (t + 1) * cht] = dloc_flat.reshape(
                cht, P
            ).T
    return ix_all, dcol_all.astype(BF16)


def feat_perm(cfg):
    """f-major feature order: new col f*h_+h holds old col h*fh+f."""
    h_, fh = cfg["h"], cfg["fh"]
    j = np.arange(h_ * fh)
    return (j % h_) * fh + j // h_


def fold_weights(inp, cfg):
    """Host-folded constant tensors (shared across cores)."""
    h_, fh, hf, mlp = cfg["h"], cfg["fh"], cfg["hf"], cfg["mlp"]
    cht = cfg["cht"]
    perm = feat_perm(cfg)

    def wa(W, a):
        # Wa[k, head] = sum_f W[k, head*fh+f] * a[head, f]
        return np.einsum("khf,hf->kh", W.reshape(-1, h_, fh), a)

    def colp(Wa):
        # permute the feature block of the 136 columns to f-major
        return np.concatenate([Wa[:, :hf][:, perm], Wa[:, hf:]], 1)

    W1, W2, W3 = inp["W1"], inp["W2"], inp["W3"]
    s1 = inp["g1"] / np.sqrt(1.0 + BN_EPS)
    t1 = inp["be1"]
    s2 = inp["g2"] / np.sqrt(1.0 + BN_EPS)
    t2 = inp["be2"]
    W2a_raw = np.concatenate([W2, wa(W2, inp["a2s"]), wa(W2, inp["a2d"])], 1)
    W2a = colp(W2a_raw * s2[:, None])
    cshift1 = colp((t2 @ W2a_raw)[None, :])[0]  # [136]
    W3a = colp(
        np.concatenate([W3, wa(W3, inp["a3s"]), wa(W3, inp["a3d"])], 1)
    )[perm, :]  # rows: layer-1 activations are f-major
    M2wp = inp["M2w"] * s1[:, None]
    mlpb2 = inp["M2b"] + t1 @ inp["M2w"]

    def bc(v, rows=P):
        return np.tile(np.asarray(v, np.float32)[None, :], (rows, 1))

    consts = dict(
        W2a=W2a.astype(BF16),
        W3a=W3a.astype(BF16),
        M1w=inp["M1w"][perm, :].astype(BF16),
        M2wp=M2wp.astype(BF16),
        b1_bc=bc(inp["b1"][perm]).astype(BF16),
        b2_bc=bc(inp["b2"][perm]).astype(BF16),
        b3_bc=bc(inp["b3"]),
        m1b_r=inp["M1b"][None, :].astype(BF16),
        m2b_r=mlpb2[None, :].astype(BF16),
        csh1_r=cshift1[None, :].astype(BF16),
        zeps_r=np.concatenate(
            [np.zeros(hf), np.full(h_, 1e-16)]
        )[None, :].astype(BF16),
        ones_r=np.ones((1, P), np.float32).astype(BF16),
        identf=np.eye(P, dtype=np.float32),
        dmaj=np.repeat(np.arange(P, dtype=np.float32), cht)[None, :]
        .repeat(P, 0)
        .astype(BF16),
        ident=np.eye(P, dtype=np.float32).astype(BF16),
    )
    return consts


# ---------------------------------------------------------------- program


def build_program(cfg):
    import concourse.bacc as bacc
    import concourse.mybir as mybir
    import concourse.tile as tile

    fp32 = mybir.dt.float32
    bf16 = mybir.dt.bfloat16
    i16 = mybir.dt.int16
    AF = mybir.ActivationFunctionType
    OP = mybir.AluOpType

    n_cores = cfg["n_cores"]
    npad, nc_nodes, tiles = cfg["npad"], cfg["nc_nodes"], cfg["tiles"]
    nsplit = cfg["nsplit"]
    ch_lo, ch_hi, cht = cfg["ch_lo"], cfg["ch_hi"], cfg["cht"]
    hf, mlp, h_, fh = cfg["hf"], cfg["mlp"], cfg["h"], cfg["fh"]
    ecols = hf + 2 * h_  # 136
    w_lo, w_hi = ch_lo * 8, ch_hi * 8
    wt = w_lo + w_hi
    ah = -(-cht // 2)  # chunks per A-transpose half

    nc = bacc.Bacc("TRN2", target_bir_lowering=False, debug=False)

    # ---- I/O
    table0_d = nc.dram_tensor("table0", [npad, ROW], bf16, kind="ExternalInput")
    ix_d = nc.dram_tensor("ix_all", [P, tiles * wt], i16, kind="ExternalInput")
    dcol_d = nc.dram_tensor(
        "dcol_all", [P, tiles * cht], bf16, kind="ExternalInput"
    )
    edst0_d = nc.dram_tensor(
        "edst0", [P, tiles * h_], bf16, kind="ExternalInput"
    )
    cn = {}
    cshapes = dict(
        W2a=[P, ecols], W3a=[P, ecols], M1w=[P, mlp], M2wp=[mlp, hf],
        b1_bc=[P, hf], b2_bc=[P, hf], b3_bc=[P, fh],
        m1b_r=[1, mlp], m2b_r=[1, hf], csh1_r=[1, ecols], ones_r=[1, P],
        zeps_r=[1, hf + h_],
        dmaj=[P, P * cht], ident=[P, P], identf=[P, P],
    )
    cdt = dict(b3_bc=fp32, identf=fp32)
    for k, shp in cshapes.items():
        cn[k] = nc.dram_tensor(k, shp, cdt.get(k, bf16), kind="ExternalInput")
    out_d = nc.dram_tensor("out", [nc_nodes, fh], fp32, kind="ExternalOutput")

    # ---- internal DRAM
    aspace = "Shared" if n_cores > 4 else "Local"
    table = [
        table0_d,
        nc.dram_tensor("table1", [npad, ROW], bf16, addr_space=aspace),
        nc.dram_tensor("table2", [npad, ROW], bf16, addr_space=aspace),
    ]
    ag_in = [
        None,
        nc.dram_tensor("ag_in1", [nc_nodes, ROW], bf16),
        nc.dram_tensor("ag_in2", [nc_nodes, ROW], bf16),
    ]

    with tile.TileContext(nc) as tc:
        with (
            tc.tile_pool(name="const", bufs=1) as cpool,
            tc.tile_pool(name="work", bufs=4) as wpool,
            tc.tile_pool(name="gath", bufs=3) as gpool,
            tc.tile_pool(name="onehot", bufs=3) as opool,
            tc.tile_pool(name="psum", bufs=2, space="PSUM") as ppool,
            tc.tile_pool(name="psumd", bufs=2, space="PSUM") as pdpool,
            tc.tile_pool(name="psumA", bufs=1, space="PSUM") as papool,
        ):
            C = {}
            for k, shp in cshapes.items():
                tl = cpool.tile(shp, cdt.get(k, bf16), tag=f"c_{k}")
                nc.sync.dma_start(out=tl[:], in_=cn[k][:, :])
                C[k] = tl

            # persistent per-layer-invariant buffers (loaded once)
            ix_sb = cpool.tile([P, tiles * wt], i16, tag="ix_sb")
            nc.sync.dma_start(out=ix_sb[:], in_=ix_d[:, :])
            dcol_sb = cpool.tile([P, tiles * cht], bf16, tag="dcol_sb")
            nc.sync.dma_start(out=dcol_sb[:], in_=dcol_d[:, :])
            edst_sb0 = cpool.tile([P, tiles * h_], bf16, tag="edst0")
            edst_sb1 = cpool.tile([P, tiles * h_], bf16, tag="edst1")
            edst_sb2 = cpool.tile([P, tiles * h_], bf16, tag="edst2")
            edst_sb = [edst_sb0, edst_sb1, edst_sb2]
            nc.sync.dma_start(out=edst_sb[0][:], in_=edst0_d[:, :])

            dmaj3 = C["dmaj"][:].rearrange("p (d c) -> p d c", c=cht)

            def fused_layer(layer):
                """Edge phase of `layer` + fused dense phase of layer+1."""
                tbl = table[layer]
                write_out = layer == 2
                for t in range(tiles):
                    gat = gpool.tile([P, cht * ROW], bf16, tag="gat")
                    g3 = gat[:].rearrange("p (c r) -> p c r", r=ROW)

                    def emit_gathers(base_c, nch_half, tbl_ap, ixoff):
                        done = 0
                        while done < nch_half:
                            gsz = min(GMAX, nch_half - done)
                            nc.gpsimd.dma_gather(
                                out_ap=g3[
                                    :, base_c + done : base_c + done + gsz, :
                                ],
                                in_ap=tbl_ap,
                                idxs_ap=ix_sb[
                                    :,
                                    ixoff + done * 8 : ixoff + (done + gsz) * 8,
                                ],
                                num_idxs=gsz * P,
                                num_idxs_reg=gsz * P,
                                elem_size=ROW,
                            )
                            done += gsz

                    emit_gathers(0, ch_lo, tbl[0:nsplit, :], t * wt)
                    emit_gathers(
                        ch_lo, ch_hi, tbl[nsplit:npad, :], t * wt + w_lo
                    )
                    # one-hot B[e, (d,c)]: c-last layout keeps every AP's
                    # last dim packed -> DVE 2x mode
                    B = opool.tile([P, P * cht], bf16, tag="B")
                    B3 = B[:].rearrange("p (d c) -> p d c", c=cht)
                    nc.vector.tensor_tensor(
                        out=B3,
                        in0=dcol_sb[:, t * cht : (t + 1) * cht][
                            :, None, :
                        ].to_broadcast([P, P, cht]),
                        in1=dmaj3,
                        op=OP.is_equal,
                    )
                    # A[d, (c,e)] = B^T per chunk, via PE transposes
                    # evacuated from PSUM by the Activation engine
                    A = opool.tile([P, cht * P], bf16, tag="A")
                    for half in range(2):
                        c0, c1 = half * ah, min((half + 1) * ah, cht)
                        pT = papool.tile([P, ah * P], bf16, tag="pT")
                        for c in range(c0, c1):
                            nc.tensor.transpose(
                                out=pT[:, (c - c0) * P : (c - c0 + 1) * P],
                                in_=B3[:, :, c],
                                identity=C["ident"][:],
                            )
                        nc.scalar.activation(
                            out=A[:, c0 * P : c1 * P],
                            in_=pT[:, : (c1 - c0) * P],
                            func=AF.Copy,
                        )
                    # e_dst expansion + leaky_relu + exp -> w in esrc slot
                    edt = edst_sb[layer][:, t * h_ : (t + 1) * h_]
                    ngrp = -(-cht // GB)
                    for g in range(ngrp):
                        c0, c1 = g * GB, min((g + 1) * GB, cht)
                        nch = c1 - c0
                        pex = ppool.tile([P, GB * h_], fp32, tag="pexp")
                        for c in range(c0, c1):
                            nc.tensor.matmul(
                                pex[:, (c - c0) * h_ : (c - c0 + 1) * h_],
                                lhsT=A[:, c * P : (c + 1) * P],
                                rhs=edt,
                                start=True,
                                stop=True,
                            )
                        u = wpool.tile([P, GB * h_], fp32, tag="u")
                        nc.vector.tensor_tensor(
                            out=u[:, : nch * h_],
                            in0=pex[:, : nch * h_],
                            in1=g3[:, c0:c1, hf : hf + h_],
                            op=OP.add,
                        )
                        ul = wpool.tile([P, GB * h_], fp32, tag="ul")
                        nc.vector.tensor_scalar(
                            out=ul[:, : nch * h_], in0=u[:, : nch * h_],
                            scalar1=NEG_SLOPE, scalar2=None, op0=OP.mult,
                        )
                        nc.vector.tensor_tensor(
                            out=u[:, : nch * h_], in0=u[:, : nch * h_],
                            in1=ul[:, : nch * h_], op=OP.max,
                        )
                        nc.scalar.activation(
                            out=g3[:, c0:c1, hf : hf + h_],
                            in_=u[:, : nch * h_],
                            func=AF.Exp,
                        )
                    # w-scale features in place (f-major rows: h is the
                    # packed last dim so the broadcast stays in 2x mode)
                    nc.vector.tensor_tensor(
                        out=g3[:, :, 0:hf].rearrange(
                            "p c (f h) -> p c f h", h=h_
                        ),
                        in0=g3[:, :, 0:hf].rearrange(
                            "p c (f h) -> p c f h", h=h_
                        ),
                        in1=g3[:, :, hf : hf + h_][:, :, None, :].to_broadcast(
                            [P, cht, fh, h_]
                        ),
                        op=OP.mult,
                    )
                    # scatter w*h (and w, for the softmax denominator);
                    # psum pre-seeded with the 1e-16 softmax-denominator
                    # epsilon via a 1-row bias matmul
                    pm = ppool.tile([P, hf + h_], fp32, tag="pmain")
                    nc.tensor.matmul(
                        pm[:], lhsT=C["ones_r"][:], rhs=C["zeps_r"][:],
                        start=True, stop=False,
                    )
                    for c in range(cht):
                        nc.tensor.matmul(
                            pm[:],
                            lhsT=B3[:, :, c],
                            rhs=g3[:, c, 0 : hf + h_],
                            start=False,
                            stop=(c == cht - 1),
                        )
                    # epilogue: normalize
                    rec = wpool.tile([P, h_], fp32, tag="rec")
                    nc.vector.reciprocal(out=rec[:], in_=pm[:, hf : hf + h_])
                    if write_out:
                        nc.vector.tensor_scalar(
                            out=rec[:], in0=rec[:], scalar1=1.0 / h_,
                            scalar2=None, op0=OP.mult,
                        )
                        fn = wpool.tile([P, hf], fp32, tag="fnw")
                        nc.vector.tensor_tensor(
                            out=fn[:].rearrange("p (f h) -> p f h", h=h_),
                            in0=pm[:, 0:hf].rearrange("p (f h) -> p f h", h=h_),
                            in1=rec[:, None, :].to_broadcast([P, fh, h_]),
                            op=OP.mult,
                        )
                        hm = wpool.tile([P, fh], fp32, tag="hm")
                        nc.vector.tensor_reduce(
                            out=hm[:],
                            in_=fn[:].rearrange("p (f h) -> p f h", h=h_),
                            axis=mybir.AxisListType.X,
                            op=OP.add,
                        )
                        ob = wpool.tile([P, fh], fp32, tag="ob")
                        nc.vector.tensor_tensor(
                            out=ob[:], in0=hm[:], in1=C["b3_bc"][:], op=OP.add
                        )
                        nc.sync.dma_start(
                            out=out_d[t * P : (t + 1) * P, :], in_=ob[:]
                        )
                        continue
                    fn = wpool.tile([P, hf], bf16, tag="fn")
                    nc.vector.tensor_tensor(
                        out=fn[:].rearrange("p (f h) -> p f h", h=h_),
                        in0=pm[:, 0:hf].rearrange("p (f h) -> p f h", h=h_),
                        in1=rec[:, None, :].to_broadcast([P, fh, h_]),
                        op=OP.mult,
                    )
                    bb = C["b1_bc"] if layer == 0 else C["b2_bc"]
                    nc.vector.tensor_tensor(
                        out=fn[:], in0=fn[:], in1=bb[:], op=OP.add
                    )
                    xo = wpool.tile([P, hf], fp32, tag="xo")
                    nc.scalar.activation(out=xo[:], in_=fn[:], func=AF.Relu)

                    # ---- fused dense phase for layer+1 on this tile
                    # (dense PSUM stages rotate through one fp32 tag)
                    pxt = pdpool.tile([P, ecols], fp32, tag="pd")
                    nc.tensor.transpose(
                        out=pxt[:, 0:P], in_=xo[:], identity=C["identf"][:]
                    )
                    xoT = wpool.tile([P, P], bf16, tag="xoT")
                    nc.vector.tensor_copy(out=xoT[:], in_=pxt[:, 0:P])
                    if layer == 0:
                        p1 = pdpool.tile([P, ecols], fp32, tag="pd")
                        nc.tensor.matmul(
                            p1[:, 0:mlp], lhsT=C["ones_r"][:],
                            rhs=C["m1b_r"][:], start=True, stop=False,
                        )
                        nc.tensor.matmul(
                            p1[:, 0:mlp], lhsT=xoT[:], rhs=C["M1w"][:],
                            start=False, stop=True,
                        )
                        r1 = wpool.tile([P, mlp], fp32, tag="r1")
                        nc.scalar.activation(
                            out=r1[:], in_=p1[:, 0:mlp], func=AF.Relu
                        )
                        pr = pdpool.tile([P, ecols], fp32, tag="pd")
                        nc.tensor.transpose(
                            out=pr[0:mlp, 0:P], in_=r1[:],
                            identity=C["identf"][:],
                        )
                        r1T = wpool.tile([mlp, P], bf16, tag="r1T")
                        nc.vector.tensor_copy(
                            out=r1T[:], in_=pr[0:mlp, 0:P]
                        )
                        p2 = pdpool.tile([P, ecols], fp32, tag="pd")
                        nc.tensor.matmul(
                            p2[:, 0:hf], lhsT=C["ones_r"][:],
                            rhs=C["m2b_r"][:], start=True, stop=False,
                        )
                        nc.tensor.matmul(
                            p2[:, 0:hf], lhsT=r1T[:], rhs=C["M2wp"][:],
                            start=False, stop=True,
                        )
                        y2 = wpool.tile([P, hf], fp32, tag="y2")
                        nc.scalar.activation(
                            out=y2[:], in_=p2[:, 0:hf], func=AF.Relu
                        )
                        py = pdpool.tile([P, ecols], fp32, tag="pd")
                        nc.tensor.transpose(
                            out=py[:, 0:P], in_=y2[:], identity=C["identf"][:]
                        )
                        y2T = wpool.tile([P, P], bf16, tag="y2T")
                        nc.vector.tensor_copy(out=y2T[:], in_=py[:, 0:P])
                        pd = pdpool.tile([P, ecols], fp32, tag="pd")
                        nc.tensor.matmul(
                            pd[:], lhsT=C["ones_r"][:], rhs=C["csh1_r"][:],
                            start=True, stop=False,
                        )
                        nc.tensor.matmul(
                            pd[:], lhsT=y2T[:], rhs=C["W2a"][:],
                            start=False, stop=True,
                        )
                    else:
                        pd = pdpool.tile([P, ecols], fp32, tag="pd")
                        nc.tensor.matmul(
                            pd[:], lhsT=xoT[:], rhs=C["W3a"][:],
                            start=True, stop=True,
                        )
                    stg = wpool.tile([P, ecols], bf16, tag="stg")
                    nc.scalar.activation(out=stg[:], in_=pd[:], func=AF.Copy)
                    nc.vector.tensor_copy(
                        out=edst_sb[layer + 1][:, t * h_ : (t + 1) * h_],
                        in_=stg[:, hf + h_ : hf + 2 * h_],
                    )
                    nc.sync.dma_start(
                        out=ag_in[layer + 1][t * P : (t + 1) * P, 0:ecols],
                        in_=stg[:],
                    )
                if not write_out:
                    nc.gpsimd.collective_compute(
                        "AllGather",
                        mybir.AluOpType.bypass,
                        ins=[ag_in[layer + 1].ap().opt()],
                        outs=[table[layer + 1].ap().opt()],
                        replica_groups=[list(range(n_cores))],
                    )

            fused_layer(0)
            fused_layer(1)
            fused_layer(2)

    nc.compile()
    return nc


# ---------------------------------------------------------------- kernel()

_CACHE = {}


def make_in_maps(inputs, cfg):
    n, npad, n_cores = cfg["n"], cfg["npad"], cfg["n_cores"]
    nc_nodes, tiles = cfg["nc_nodes"], cfg["tiles"]
    h_ = cfg["h"]
    inp = {k: np.asarray(v) for k, v in inputs.items()}
    ix_all, dcol_all = preprocess_edges(inp["edge_index"], cfg)
    consts = fold_weights(inp, cfg)
    xpad = np.zeros((npad, cfg["fin"]), np.float32)
    xpad[:n] = inp["x"]

    def wa(W, a):
        return np.einsum(
            "khf,hf->kh", W.reshape(-1, cfg["h"], cfg["fh"]), a
        )

    # host-computed layer-0 node table: [h (f-major) | e_src | e_dst | 0-pad]
    hf = cfg["hf"]
    table0 = np.zeros((npad, ROW), np.float32)
    table0[:, :hf] = (xpad @ inp["W1"])[:, feat_perm(cfg)]
    table0[:, hf : hf + h_] = xpad @ wa(inp["W1"], inp["a1s"])
    table0[:, hf + h_ : hf + 2 * h_] = xpad @ wa(inp["W1"], inp["a1d"])
    table0 = table0.astype(BF16)
    in_maps = []
    for c in range(n_cores):
        # edst0 in [P, tiles*h] partition-major layout for own nodes
        ed = table0[c * nc_nodes : (c + 1) * nc_nodes, hf + h_ : hf + 2 * h_]
        ed_pm = np.ascontiguousarray(
            ed.reshape(tiles, P, h_).transpose(1, 0, 2).reshape(P, tiles * h_)
        )
        m = dict(
            table0=table0,
            ix_all=ix_all[c],
            dcol_all=dcol_all[c],
            edst0=ed_pm,
        )
        for k, v in consts.items():
            m[k] = v
        in_maps.append(m)
    return in_maps


def kernel(**inputs):
    from concourse import bass_utils

    cfg = derive_cfg(full_cfg())
    in_maps = make_in_maps(inputs, cfg)
    key = ("prog", cfg["ch_lo"], cfg["ch_hi"], cfg["nsplit"])
    if key not in _CACHE:
        _CACHE[key] = build_program(cfg)
    nc = _CACHE[key]
    res = bass_utils.run_bass_kernel_spmd(
        nc, in_maps, core_ids=list(range(cfg["n_cores"]))
    )
    outs = [res.results[c]["out"] for c in range(cfg["n_cores"])]
    full = np.concatenate(outs, axis=0)[: cfg["n"]]
    return full.astype(np.float32)


# revision 34
# speedup vs baseline: 2.0346x; 1.6815x over previous
"""EnhancedGAT Trainium2 kernel: 8-core SPMD, node-sharded edge phase.

Design:
- Nodes padded to NPAD (multiple of 8*128); core c owns dst nodes
  [c*NC_NODES, (c+1)*NC_NODES), processed as TILES tiles of 128 dst nodes.
- Edges (+self loops) sorted by dst, assigned to the owning core/tile,
  split per tile into lo/hi halves by src (< NSPLIT, chosen so the lo
  half fits one <=1024-index dma_gather) so gather indices fit int16,
  padded to uniform chunk counts CH_LO/CH_HI across all cores/tiles.
- Layer-0 node table (dense phase of layer 0) is computed on the host
  and shipped as an input; layers 1/2 compute own table rows fused into
  the previous edge phase's tile loop, then AllGather.
- Feature columns are kept in f-major (f*H+h) order end-to-end so the
  per-edge softmax-weight scaling is a packed-last-dim DVE op (2x mode).
- Edge phase per tile: dma_gather of src rows; one-hot B[e,(d,c)] built
  on DVE in c-last layout (2x mode); A[d,(c,e)] = per-chunk PE
  transposes of B evacuated by the Activation engine; per-chunk one-hot
  matmuls scatter w*h and w into PSUM; e_dst per edge via small PE
  matmuls with A; leaky-relu/exp on the Activation engine.
"""

import sys

sys.path.insert(0, "/opt/trn_rl_repo")

import numpy as np
import ml_dtypes

BF16 = ml_dtypes.bfloat16
NEG_SLOPE = 0.2
BN_EPS = 1e-5
P = 128
ROW = 256  # bf16 cols per table row (512B)
GB = 24  # chunks per u-add group
GMAX = 8  # max chunks per dma_gather call (1024-descriptor hw limit)


def full_cfg():
    return dict(
        n_cores=8, n=50000, e=800000, fin=128, h=4, fh=32, hf=128, mlp=64
    )


def derive_cfg(cfg):
    c = dict(cfg)
    n_cores = c["n_cores"]
    npad = ((c["n"] + n_cores * P - 1) // (n_cores * P)) * (n_cores * P)
    c["npad"] = npad
    c["nc_nodes"] = npad // n_cores
    c["tiles"] = c["nc_nodes"] // P
    return c


# ---------------------------------------------------------------- host side


def _wrap_idx(flat):
    """int16 gather-index layout: flat[i] lives at wrapped[i%16, i//16],
    replicated to 128 partitions."""
    s = len(flat) // 16
    w = flat.reshape(s, 16).T.astype(np.int16)  # [16, s]
    return np.tile(w, (P // 16, 1))  # [128, s]


def preprocess_edges(edge_index, cfg):
    """Returns per-core index arrays + updates cfg with CH_LO/CH_HI."""
    n, npad, n_cores = cfg["n"], cfg["npad"], cfg["n_cores"]
    nc_nodes, tiles = cfg["nc_nodes"], cfg["tiles"]
    src = np.concatenate([edge_index[0], np.arange(n, dtype=np.int64)])
    dst = np.concatenate([edge_index[1], np.arange(n, dtype=np.int64)])
    order = np.argsort(dst, kind="stable")
    src_s = src[order].astype(np.int32)
    dst_s = dst[order].astype(np.int32)

    # pick the lo/hi split (both halves must index with int16, i.e. be
    # <= 32768 rows) minimizing dma_gather calls then total chunks
    bounds = np.searchsorted(dst_s, np.arange(0, npad + 1, P))
    nblk = npad // P
    ntile = n_cores * tiles
    tile_of_edge = np.searchsorted(bounds, np.arange(len(dst_s)), "right") - 1
    srcblk = src_s // P
    hist = np.zeros((ntile, nblk + 1), np.int64)
    np.add.at(hist, (tile_of_edge, srcblk), 1)
    pref = np.cumsum(hist, axis=1)  # pref[ti, k] = edges with src < (k+1)*P
    tot = pref[:, -1]
    lo_blk = max(1, (npad - 32768) // P)
    hi_blk = min(32768 // P, nblk)
    best = None
    for k in range(lo_blk, hi_blk + 1):
        max_lo = int(pref[:, k - 1].max())
        max_hi = int((tot - pref[:, k - 1]).max())
        cl = max(1, -(-max_lo // P))
        ch = max(1, -(-max_hi // P))
        key = (-(-cl // GMAX) + -(-ch // GMAX), cl + ch)
        if best is None or key < best[0]:
            best = (key, k * P, cl, ch)
    _, nsplit, ch_lo, ch_hi = best
    cfg["nsplit"] = nsplit

    per_ct = []  # (src_lo, dloc_lo, src_hi, dloc_hi) per (core,tile)
    for c in range(n_cores):
        for t in range(tiles):
            ti = c * tiles + t
            a, b = bounds[ti], bounds[ti + 1]
            srcs = src_s[a:b]
            dloc = (dst_s[a:b] - ti * P).astype(np.int32)
            m = srcs < nsplit
            sl, dl = srcs[m], dloc[m]
            sh, dh = srcs[~m] - nsplit, dloc[~m]
            per_ct.append((sl, dl, sh, dh))
    cht = ch_lo + ch_hi
    cfg["ch_lo"], cfg["ch_hi"], cfg["cht"] = ch_lo, ch_hi, cht

    w_lo, w_hi = ch_lo * 8, ch_hi * 8
    wt = w_lo + w_hi
    ix_all = np.zeros((n_cores, P, tiles * wt), np.int16)
    dcol_all = np.full((n_cores, P, tiles * cht), -1.0, np.float32)
    for c in range(n_cores):
        for t in range(tiles):
            sl, dl, sh, dh = per_ct[c * tiles + t]
            fl = np.zeros(ch_lo * P, np.int32)
            fl[: len(sl)] = sl
            fhh = np.zeros(ch_hi * P, np.int32)
            fhh[: len(sh)] = sh
            ix_all[c, :, t * wt : t * wt + w_lo] = _wrap_idx(fl)
            ix_all[c, :, t * wt + w_lo : (t + 1) * wt] = _wrap_idx(fhh)
            # dcol[p, c] = dst-local index of edge slot (chunk c, lane p)
            dloc_flat = np.full(cht * P, -1.0, np.float32)
            dloc_flat[: len(dl)] = dl
            dloc_flat[ch_lo * P : ch_lo * P + len(dh)] = dh
            dcol_all[c, :, t * cht : (t + 1) * cht] = dloc_flat.reshape(
                cht, P
            ).T
    return ix_all, dcol_all.astype(BF16)


def feat_perm(cfg):
    """f-major feature order: new col f*h_+h holds old col h*fh+f."""
    h_, fh = cfg["h"], cfg["fh"]
    j = np.arange(h_ * fh)
    return (j % h_) * fh + j // h_


def fold_weights(inp, cfg):
    """Host-folded constant tensors (shared across cores)."""
    h_, fh, hf, mlp = cfg["h"], cfg["fh"], cfg["hf"], cfg["mlp"]
    cht = cfg["cht"]
    perm = feat_perm(cfg)

    def wa(W, a):
        # Wa[k, head] = sum_f W[k, head*fh+f] * a[head, f]
        return np.einsum("khf,hf->kh", W.reshape(-1, h_, fh), a)

    def colp(Wa):
        # permute the feature block of the 136 columns to f-major
        return np.concatenate([Wa[:, :hf][:, perm], Wa[:, hf:]], 1)

    W1, W2, W3 = inp["W1"], inp["W2"], inp["W3"]
    s1 = inp["g1"] / np.sqrt(1.0 + BN_EPS)
    t1 = inp["be1"]
    s2 = inp["g2"] / np.sqrt(1.0 + BN_EPS)
    t2 = inp["be2"]
    W2a_raw = np.concatenate([W2, wa(W2, inp["a2s"]), wa(W2, inp["a2d"])], 1)
    W2a = colp(W2a_raw * s2[:, None])
    cshift1 = colp((t2 @ W2a_raw)[None, :])[0]  # [136]
    W3a = colp(
        np.concatenate([W3, wa(W3, inp["a3s"]), wa(W3, inp["a3d"])], 1)
    )[perm, :]  # rows: layer-1 activations are f-major
    M2wp = inp["M2w"] * s1[:, None]
    mlpb2 = inp["M2b"] + t1 @ inp["M2w"]

    def bc(v, rows=P):
        return np.tile(np.asarray(v, np.float32)[None, :], (rows, 1))

    consts = dict(
        W2a=W2a.astype(BF16),
        W3a=W3a.astype(BF16),
        M1w=inp["M1w"][perm, :].astype(BF16),
        M2wp=M2wp.astype(BF16),
        b1_bc=bc(inp["b1"][perm]).astype(BF16),
        b2_bc=bc(inp["b2"][perm]).astype(BF16),
        b3_bc=bc(inp["b3"]),
        m1b_r=inp["M1b"][None, :].astype(BF16),
        m2b_r=mlpb2[None, :].astype(BF16),
        csh1_r=cshift1[None, :].astype(BF16),
        zeps_r=np.concatenate(
            [np.zeros(hf), np.full(h_, 1e-16)]
        )[None, :].astype(BF16),
        ones_r=np.ones((1, P), np.float32).astype(BF16),
        identf=np.eye(P, dtype=np.float32),
        dmaj=np.repeat(np.arange(P, dtype=np.float32), cht)[None, :]
        .repeat(P, 0)
        .astype(BF16),
        ident=np.eye(P, dtype=np.float32).astype(BF16),
    )
    return consts


# ---------------------------------------------------------------- program


def build_program(cfg):
    import concourse.bacc as bacc
    import concourse.mybir as mybir
    import concourse.tile as tile

    fp32 = mybir.dt.float32
    bf16 = mybir.dt.bfloat16
    i16 = mybir.dt.int16
    AF = mybir.ActivationFunctionType
    OP = mybir.AluOpType

    n_cores = cfg["n_cores"]
    npad, nc_nodes, tiles = cfg["npad"], cfg["nc_nodes"], cfg["tiles"]
    nsplit = cfg["nsplit"]
    ch_lo, ch_hi, cht = cfg["ch_lo"], cfg["ch_hi"], cfg["cht"]
    hf, mlp, h_, fh = cfg["hf"], cfg["mlp"], cfg["h"], cfg["fh"]
    ecols = hf + 2 * h_  # 136
    w_lo, w_hi = ch_lo * 8, ch_hi * 8
    wt = w_lo + w_hi
    ah = -(-cht // 2)  # chunks per A-transpose half

    nc = bacc.Bacc("TRN2", target_bir_lowering=False, debug=False)

    # ---- I/O
    table0_d = nc.dram_tensor("table0", [npad, ROW], bf16, kind="ExternalInput")
    ix_d = nc.dram_tensor("ix_all", [P, tiles * wt], i16, kind="ExternalInput")
    dcol_d = nc.dram_tensor(
        "dcol_all", [P, tiles * cht], bf16, kind="ExternalInput"
    )
    edst0_d = nc.dram_tensor(
        "edst0", [P, tiles * h_], bf16, kind="ExternalInput"
    )
    cn = {}
    cshapes = dict(
        W2a=[P, ecols], W3a=[P, ecols], M1w=[P, mlp], M2wp=[mlp, hf],
        b1_bc=[P, hf], b2_bc=[P, hf], b3_bc=[P, fh],
        m1b_r=[1, mlp], m2b_r=[1, hf], csh1_r=[1, ecols], ones_r=[1, P],
        zeps_r=[1, hf + h_],
        dmaj=[P, P * cht], ident=[P, P], identf=[P, P],
    )
    cdt = dict(b3_bc=fp32, identf=fp32)
    for k, shp in cshapes.items():
        cn[k] = nc.dram_tensor(k, shp, cdt.get(k, bf16), kind="ExternalInput")
    out_d = nc.dram_tensor("out", [nc_nodes, fh], fp32, kind="ExternalOutput")

    # ---- internal DRAM
    aspace = "Shared" if n_cores > 4 else "Local"
    table = [
        table0_d,
        nc.dram_tensor("table1", [npad, ROW], bf16, addr_space=aspace),
        nc.dram_tensor("table2", [npad, ROW], bf16, addr_space=aspace),
    ]
    gcols = hf + h_  # 132: only [feat | e_src] is gathered/exchanged
    ag_in = [
        None,
        nc.dram_tensor("ag_in1", [nc_nodes, gcols], bf16),
        nc.dram_tensor("ag_in2", [nc_nodes, gcols], bf16),
    ]

    with tile.TileContext(nc) as tc:
        with (
            tc.tile_pool(name="const", bufs=1) as cpool,
            tc.tile_pool(name="work", bufs=4) as wpool,
            tc.tile_pool(name="gath", bufs=3) as gpool,
            tc.tile_pool(name="onehot", bufs=3) as opool,
            tc.tile_pool(name="psum", bufs=2, space="PSUM") as ppool,
            tc.tile_pool(name="psumd", bufs=2, space="PSUM") as pdpool,
            tc.tile_pool(name="psumA", bufs=1, space="PSUM") as papool,
        ):
            C = {}
            for k, shp in cshapes.items():
                tl = cpool.tile(shp, cdt.get(k, bf16), tag=f"c_{k}")
                nc.sync.dma_start(out=tl[:], in_=cn[k][:, :])
                C[k] = tl

            # persistent per-layer-invariant buffers (loaded once)
            ix_sb = cpool.tile([P, tiles * wt], i16, tag="ix_sb")
            nc.sync.dma_start(out=ix_sb[:], in_=ix_d[:, :])
            dcol_sb = cpool.tile([P, tiles * cht], bf16, tag="dcol_sb")
            nc.sync.dma_start(out=dcol_sb[:], in_=dcol_d[:, :])
            edst_sb0 = cpool.tile([P, tiles * h_], bf16, tag="edst0")
            edst_sb1 = cpool.tile([P, tiles * h_], bf16, tag="edst1")
            edst_sb2 = cpool.tile([P, tiles * h_], bf16, tag="edst2")
            edst_sb = [edst_sb0, edst_sb1, edst_sb2]
            nc.sync.dma_start(out=edst_sb[0][:], in_=edst0_d[:, :])

            dmaj3 = C["dmaj"][:].rearrange("p (d c) -> p d c", c=cht)

            def fused_layer(layer):
                """Edge phase of `layer` + fused dense phase of layer+1."""
                tbl = table[layer]
                write_out = layer == 2
                for t in range(tiles):
                    gat = gpool.tile([P, cht * ROW], bf16, tag="gat")
                    g3 = gat[:].rearrange("p (c r) -> p c r", r=ROW)

                    def emit_gathers(base_c, nch_half, tbl_ap, ixoff):
                        done = 0
                        while done < nch_half:
                            gsz = min(GMAX, nch_half - done)
                            nc.gpsimd.dma_gather(
                                out_ap=g3[
                                    :, base_c + done : base_c + done + gsz, :
                                ],
                                in_ap=tbl_ap,
                                idxs_ap=ix_sb[
                                    :,
                                    ixoff + done * 8 : ixoff + (done + gsz) * 8,
                                ],
                                num_idxs=gsz * P,
                                num_idxs_reg=gsz * P,
                                elem_size=ROW,
                            )
                            done += gsz

                    emit_gathers(0, ch_lo, tbl[0:nsplit, :], t * wt)
                    emit_gathers(
                        ch_lo, ch_hi, tbl[nsplit:npad, :], t * wt + w_lo
                    )
                    # one-hot B[e, (d,c)]: c-last layout keeps every AP's
                    # last dim packed -> DVE 2x mode
                    B = opool.tile([P, P * cht], bf16, tag="B")
                    B3 = B[:].rearrange("p (d c) -> p d c", c=cht)
                    nc.vector.tensor_tensor(
                        out=B3,
                        in0=dcol_sb[:, t * cht : (t + 1) * cht][
                            :, None, :
                        ].to_broadcast([P, P, cht]),
                        in1=dmaj3,
                        op=OP.is_equal,
                    )
                    # A[d, (c,e)] = B^T per chunk, via PE transposes
                    # evacuated from PSUM by the Activation engine
                    A = opool.tile([P, cht * P], bf16, tag="A")
                    for half in range(2):
                        c0, c1 = half * ah, min((half + 1) * ah, cht)
                        pT = papool.tile([P, ah * P], bf16, tag="pT")
                        for c in range(c0, c1):
                            nc.tensor.transpose(
                                out=pT[:, (c - c0) * P : (c - c0 + 1) * P],
                                in_=B3[:, :, c],
                                identity=C["ident"][:],
                            )
                        nc.scalar.activation(
                            out=A[:, c0 * P : c1 * P],
                            in_=pT[:, : (c1 - c0) * P],
                            func=AF.Copy,
                        )
                    # e_dst expansion + leaky_relu + exp -> w in esrc slot
                    edt = edst_sb[layer][:, t * h_ : (t + 1) * h_]
                    ngrp = -(-cht // GB)
                    for g in range(ngrp):
                        c0, c1 = g * GB, min((g + 1) * GB, cht)
                        nch = c1 - c0
                        pex = ppool.tile([P, GB * h_], fp32, tag="pexp")
                        for c in range(c0, c1):
                            nc.tensor.matmul(
                                pex[:, (c - c0) * h_ : (c - c0 + 1) * h_],
                                lhsT=A[:, c * P : (c + 1) * P],
                                rhs=edt,
                                start=True,
                                stop=True,
                            )
                        u = wpool.tile([P, GB * h_], fp32, tag="u")
                        nc.vector.tensor_tensor(
                            out=u[:, : nch * h_],
                            in0=pex[:, : nch * h_],
                            in1=g3[:, c0:c1, hf : hf + h_],
                            op=OP.add,
                        )
                        ul = wpool.tile([P, GB * h_], fp32, tag="ul")
                        nc.vector.tensor_scalar(
                            out=ul[:, : nch * h_], in0=u[:, : nch * h_],
                            scalar1=NEG_SLOPE, scalar2=None, op0=OP.mult,
                        )
                        nc.vector.tensor_tensor(
                            out=u[:, : nch * h_], in0=u[:, : nch * h_],
                            in1=ul[:, : nch * h_], op=OP.max,
                        )
                        nc.scalar.activation(
                            out=g3[:, c0:c1, hf : hf + h_],
                            in_=u[:, : nch * h_],
                            func=AF.Exp,
                        )
                    # w-scale features in place (f-major rows: h is the
                    # packed last dim so the broadcast stays in 2x mode)
                    nc.vector.tensor_tensor(
                        out=g3[:, :, 0:hf].rearrange(
                            "p c (f h) -> p c f h", h=h_
                        ),
                        in0=g3[:, :, 0:hf].rearrange(
                            "p c (f h) -> p c f h", h=h_
                        ),
                        in1=g3[:, :, hf : hf + h_][:, :, None, :].to_broadcast(
                            [P, cht, fh, h_]
                        ),
                        op=OP.mult,
                    )
                    # scatter w*h (and w, for the softmax denominator);
                    # psum pre-seeded with the 1e-16 softmax-denominator
                    # epsilon via a 1-row bias matmul
                    pm = ppool.tile([P, hf + h_], fp32, tag="pmain")
                    nc.tensor.matmul(
                        pm[:], lhsT=C["ones_r"][:], rhs=C["zeps_r"][:],
                        start=True, stop=False,
                    )
                    for c in range(cht):
                        nc.tensor.matmul(
                            pm[:],
                            lhsT=B3[:, :, c],
                            rhs=g3[:, c, 0 : hf + h_],
                            start=False,
                            stop=(c == cht - 1),
                        )
                    # epilogue: normalize
                    rec = wpool.tile([P, h_], fp32, tag="rec")
                    nc.vector.reciprocal(out=rec[:], in_=pm[:, hf : hf + h_])
                    if write_out:
                        nc.vector.tensor_scalar(
                            out=rec[:], in0=rec[:], scalar1=1.0 / h_,
                            scalar2=None, op0=OP.mult,
                        )
                        fn = wpool.tile([P, hf], fp32, tag="fnw")
                        nc.vector.tensor_tensor(
                            out=fn[:].rearrange("p (f h) -> p f h", h=h_),
                            in0=pm[:, 0:hf].rearrange("p (f h) -> p f h", h=h_),
                            in1=rec[:, None, :].to_broadcast([P, fh, h_]),
                            op=OP.mult,
                        )
                        hm = wpool.tile([P, fh], fp32, tag="hm")
                        nc.vector.tensor_reduce(
                            out=hm[:],
                            in_=fn[:].rearrange("p (f h) -> p f h", h=h_),
                            axis=mybir.AxisListType.X,
                            op=OP.add,
                        )
                        ob = wpool.tile([P, fh], fp32, tag="ob")
                        nc.vector.tensor_tensor(
                            out=ob[:], in0=hm[:], in1=C["b3_bc"][:], op=OP.add
                        )
                        nc.sync.dma_start(
                            out=out_d[t * P : (t + 1) * P, :], in_=ob[:]
                        )
                        continue
                    fn = wpool.tile([P, hf], bf16, tag="fn")
                    nc.vector.tensor_tensor(
                        out=fn[:].rearrange("p (f h) -> p f h", h=h_),
                        in0=pm[:, 0:hf].rearrange("p (f h) -> p f h", h=h_),
                        in1=rec[:, None, :].to_broadcast([P, fh, h_]),
                        op=OP.mult,
                    )
                    bb = C["b1_bc"] if layer == 0 else C["b2_bc"]
                    nc.vector.tensor_tensor(
                        out=fn[:], in0=fn[:], in1=bb[:], op=OP.add
                    )
                    xo = wpool.tile([P, hf], fp32, tag="xo")
                    nc.scalar.activation(out=xo[:], in_=fn[:], func=AF.Relu)

                    # ---- fused dense phase for layer+1 on this tile
                    # (dense PSUM stages rotate through one fp32 tag)
                    pxt = pdpool.tile([P, ecols], fp32, tag="pd")
                    nc.tensor.transpose(
                        out=pxt[:, 0:P], in_=xo[:], identity=C["identf"][:]
                    )
                    xoT = wpool.tile([P, P], bf16, tag="xoT")
                    nc.vector.tensor_copy(out=xoT[:], in_=pxt[:, 0:P])
                    if layer == 0:
                        p1 = pdpool.tile([P, ecols], fp32, tag="pd")
                        nc.tensor.matmul(
                            p1[:, 0:mlp], lhsT=C["ones_r"][:],
                            rhs=C["m1b_r"][:], start=True, stop=False,
                        )
                        nc.tensor.matmul(
                            p1[:, 0:mlp], lhsT=xoT[:], rhs=C["M1w"][:],
                            start=False, stop=True,
                        )
                        r1 = wpool.tile([P, mlp], fp32, tag="r1")
                        nc.scalar.activation(
                            out=r1[:], in_=p1[:, 0:mlp], func=AF.Relu
                        )
                        pr = pdpool.tile([P, ecols], fp32, tag="pd")
                        nc.tensor.transpose(
                            out=pr[0:mlp, 0:P], in_=r1[:],
                            identity=C["identf"][:],
                        )
                        r1T = wpool.tile([mlp, P], bf16, tag="r1T")
                        nc.vector.tensor_copy(
                            out=r1T[:], in_=pr[0:mlp, 0:P]
                        )
                        p2 = pdpool.tile([P, ecols], fp32, tag="pd")
                        nc.tensor.matmul(
                            p2[:, 0:hf], lhsT=C["ones_r"][:],
                            rhs=C["m2b_r"][:], start=True, stop=False,
                        )
                        nc.tensor.matmul(
                            p2[:, 0:hf], lhsT=r1T[:], rhs=C["M2wp"][:],
                            start=False, stop=True,
                        )
                        y2 = wpool.tile([P, hf], fp32, tag="y2")
                        nc.scalar.activation(
                            out=y2[:], in_=p2[:, 0:hf], func=AF.Relu
                        )
                        py = pdpool.tile([P, ecols], fp32, tag="pd")
                        nc.tensor.transpose(
                            out=py[:, 0:P], in_=y2[:], identity=C["identf"][:]
                        )
                        y2T = wpool.tile([P, P], bf16, tag="y2T")
                        nc.vector.tensor_copy(out=y2T[:], in_=py[:, 0:P])
                        pd = pdpool.tile([P, ecols], fp32, tag="pd")
                        nc.tensor.matmul(
                            pd[:], lhsT=C["ones_r"][:], rhs=C["csh1_r"][:],
                            start=True, stop=False,
                        )
                        nc.tensor.matmul(
                            pd[:], lhsT=y2T[:], rhs=C["W2a"][:],
                            start=False, stop=True,
                        )
                    else:
                        pd = pdpool.tile([P, ecols], fp32, tag="pd")
                        nc.tensor.matmul(
                            pd[:], lhsT=xoT[:], rhs=C["W3a"][:],
                            start=True, stop=True,
                        )
                    stg = wpool.tile([P, ecols], bf16, tag="stg")
                    nc.scalar.activation(out=stg[:], in_=pd[:], func=AF.Copy)
                    nc.vector.tensor_copy(
                        out=edst_sb[layer + 1][:, t * h_ : (t + 1) * h_],
                        in_=stg[:, hf + h_ : hf + 2 * h_],
                    )
                    nc.sync.dma_start(
                        out=ag_in[layer + 1][t * P : (t + 1) * P, :],
                        in_=stg[:, 0:gcols],
                    )
                if not write_out:
                    nc.gpsimd.collective_compute(
                        "AllGather",
                        mybir.AluOpType.bypass,
                        ins=[ag_in[layer + 1].ap().opt()],
                        outs=[table[layer + 1].ap()[:, 0:gcols].opt()],
                        replica_groups=[list(range(n_cores))],
                    )

            fused_layer(0)
            fused_layer(1)
            fused_layer(2)

    nc.compile()
    return nc


# ---------------------------------------------------------------- kernel()

_CACHE = {}


def make_in_maps(inputs, cfg):
    n, npad, n_cores = cfg["n"], cfg["npad"], cfg["n_cores"]
    nc_nodes, tiles = cfg["nc_nodes"], cfg["tiles"]
    h_ = cfg["h"]
    inp = {k: np.asarray(v) for k, v in inputs.items()}
    ix_all, dcol_all = preprocess_edges(inp["edge_index"], cfg)
    consts = fold_weights(inp, cfg)
    xpad = np.zeros((npad, cfg["fin"]), np.float32)
    xpad[:n] = inp["x"]

    def wa(W, a):
        return np.einsum(
            "khf,hf->kh", W.reshape(-1, cfg["h"], cfg["fh"]), a
        )

    # host-computed layer-0 node table: [h (f-major) | e_src | e_dst | 0-pad]
    hf = cfg["hf"]
    table0 = np.zeros((npad, ROW), np.float32)
    table0[:, :hf] = (xpad @ inp["W1"])[:, feat_perm(cfg)]
    table0[:, hf : hf + h_] = xpad @ wa(inp["W1"], inp["a1s"])
    table0[:, hf + h_ : hf + 2 * h_] = xpad @ wa(inp["W1"], inp["a1d"])
    table0 = table0.astype(BF16)
    in_maps = []
    for c in range(n_cores):
        # edst0 in [P, tiles*h] partition-major layout for own nodes
        ed = table0[c * nc_nodes : (c + 1) * nc_nodes, hf + h_ : hf + 2 * h_]
        ed_pm = np.ascontiguousarray(
            ed.reshape(tiles, P, h_).transpose(1, 0, 2).reshape(P, tiles * h_)
        )
        m = dict(
            table0=table0,
            ix_all=ix_all[c],
            dcol_all=dcol_all[c],
            edst0=ed_pm,
        )
        for k, v in consts.items():
            m[k] = v
        in_maps.append(m)
    return in_maps


def kernel(**inputs):
    from concourse import bass_utils

    cfg = derive_cfg(full_cfg())
    in_maps = make_in_maps(inputs, cfg)
    key = ("prog", cfg["ch_lo"], cfg["ch_hi"], cfg["nsplit"])
    if key not in _CACHE:
        _CACHE[key] = build_program(cfg)
    nc = _CACHE[key]
    res = bass_utils.run_bass_kernel_spmd(
        nc, in_maps, core_ids=list(range(cfg["n_cores"]))
    )
    outs = [res.results[c]["out"] for c in range(cfg["n_cores"])]
    full = np.concatenate(outs, axis=0)[: cfg["n"]]
    return full.astype(np.float32)
